# revision 13
# baseline (speedup 1.0000x reference)
"""GQA causal attention with RoPE, tensor-parallel over heads on 8 TRN2 NeuronCores.

Reference computation (per problem spec, all f32):
  q = rope(x @ Wq), k = rope(x @ Wk), v = x @ Wv    (GQA: 32 q heads, 8 kv heads, hd=64)
  out = softmax(causal(q k^T / 8)) v @ Wo

Sharding: core c owns q-heads 4c..4c+3 and kv-head c (column shards of
Wq/Wk/Wv).  Attention outputs (kept transposed, feature-major) are
AllGathered per batch; the Wo projection is column-split: core c computes
out[:, 256c:256(c+1)] with the full gathered activations, so the final
output assembles by concatenation with no AllReduce.

Layout trick: scores are computed transposed (S^T = K Q^T, keys on
partitions, queries free) so the exp'd scores feed the PV matmul directly
as the moving operand — no P transposes.  A ones-column appended to V
yields the softmax denominators in the same PV matmul.

Compute dtype on the TensorEngine is bf16 (f32 accumulation in PSUM);
softmax runs in f32 on the scalar/vector engines.  x^T is produced by
xbar DMA transpose (bf16), keeping the TensorEngine free for matmuls.
"""

import os
import sys

import numpy as np

for _p in ("/opt/trn_rl_repo",):
    if os.path.isdir(_p) and _p not in sys.path:
        sys.path.insert(0, _p)

from contextlib import ExitStack

import concourse.bass as bass
import concourse.tile as tile
from concourse import bacc, mybir
from concourse.bass_utils import run_bass_kernel_spmd

B, S, HID = 2, 2048, 2048
NH, NKV, HD = 32, 8, 64
TP = 8
QH = NH // TP          # 4 q heads per core
T = B * S              # 4096 tokens
QF = QH * HD           # 256 q features per core
OC = HID // TP         # 256 out cols per core
TOKC = 512             # token chunk for projection / q-chunk for attention
NHB = HID // 128       # 16 hid blocks

F32 = mybir.dt.float32
BF = mybir.dt.bfloat16

LAST_RESULTS = None
_NC_CACHE = None


def build_nc():
    nc = bacc.Bacc(None, target_bir_lowering=False)

    x = nc.declare_dram_parameter("x", [T, HID], F32, False)
    cos = nc.declare_dram_parameter("cos", [S, HD], F32, False)
    sin = nc.declare_dram_parameter("sin", [S, HD], F32, False)
    wq = nc.declare_dram_parameter("Wq", [HID, QF], F32, False)
    wk = nc.declare_dram_parameter("Wk", [HID, HD], F32, False)
    wv = nc.declare_dram_parameter("Wv", [HID, HD], F32, False)
    wo = nc.declare_dram_parameter("Wo", [HID, OC], F32, False)
    out = nc.declare_dram_parameter("out", [T, OC], F32, isOutput=True)

    with tile.TileContext(nc) as tc, ExitStack() as ctx:
        const = ctx.enter_context(tc.tile_pool(name="const", bufs=1))
        dram = ctx.enter_context(tc.tile_pool(name="dram", bufs=1, space="DRAM"))

        # PSUM: 2x 2-bank score slots + 4x 1-bank slots = 8 banks
        psum_s = ctx.enter_context(tc.tile_pool(name="psum_s", bufs=2, space="PSUM"))
        psum = ctx.enter_context(tc.tile_pool(name="psum_o", bufs=4, space="PSUM"))

        # ---- constants -------------------------------------------------
        ones128 = const.tile([128, 128], BF)
        nc.vector.memset(ones128[:], 1.0)
        ident = const.tile([128, 128], BF)
        nc.gpsimd.affine_select(
            ident[:], ones128[:], pattern=[[-1, 128]], base=0,
            channel_multiplier=1, compare_op=mybir.AluOpType.is_equal, fill=0.0,
        )
        ones_col = const.tile([1, 64], BF)
        nc.vector.memset(ones_col[:], 1.0)
        id64hi = const.tile([128, 64], BF)
        nc.gpsimd.affine_select(
            id64hi[64:128, :], ones128[64:128, 0:64], pattern=[[-1, 64]], base=0,
            channel_multiplier=1, compare_op=mybir.AluOpType.is_equal, fill=0.0,
        )

        # ---- weights (bf16 casts) -------------------------------------
        wq_sb = []
        wkv_sb = []
        wo_sb = []
        for hb in range(NHB):
            t = const.tile([128, QF], BF, name=f"wq{hb}")
            nc.gpsimd.dma_start(t[:], wq[hb * 128:(hb + 1) * 128, :])
            wq_sb.append(t)
            t = const.tile([128, 128], BF, name=f"wkv{hb}")
            nc.gpsimd.dma_start(t[:, 0:HD], wk[hb * 128:(hb + 1) * 128, :])
            nc.gpsimd.dma_start(t[:, HD:128], wv[hb * 128:(hb + 1) * 128, :])
            wkv_sb.append(t)
            t = const.tile([128, OC], BF, name=f"wo{hb}")
            nc.gpsimd.dma_start(t[:], wo[hb * 128:(hb + 1) * 128, :])
            wo_sb.append(t)

        # ---- RoPE tables: cosT/sinTs [128, S] bf16 --------------------
        # rows 0..63 = cos^T (d-major); rows 64..127 duplicate (2 heads/tile)
        # sinTs rows 0..31 = -sin^T[0:32], rows 32..63 = +sin^T[32:64]
        cosT = const.tile([128, S], BF)
        sinTs = const.tile([128, S], BF)
        with tc.tile_pool(name="ropebld", bufs=4) as rb:
            for i in range(S // 128):
                cn = rb.tile([128, HD], BF, tag="cn")
                nc.gpsimd.dma_start(cn[:], cos[i * 128:(i + 1) * 128, :])
                ps = psum.tile([HD, 128], BF, tag="o", name=f"cps{i}")
                nc.tensor.transpose(ps[:], cn[:], ident[:])
                nc.scalar.copy(cosT[0:HD, i * 128:(i + 1) * 128], ps[:])
                sn = rb.tile([128, HD], BF, tag="sn")
                nc.gpsimd.dma_start(sn[:], sin[i * 128:(i + 1) * 128, :])
                ps2 = psum.tile([HD, 128], BF, tag="o", name=f"sps{i}")
                nc.tensor.transpose(ps2[:], sn[:], ident[:])
                nc.scalar.mul(sinTs[0:32, i * 128:(i + 1) * 128], ps2[0:32, :], -1.0)
                nc.scalar.copy(sinTs[32:HD, i * 128:(i + 1) * 128], ps2[32:HD, :])
        nc.gpsimd.dma_start(cosT[HD:128, :], cosT[0:HD, :])
        nc.gpsimd.dma_start(sinTs[HD:128, :], sinTs[0:HD, :])

        # ---- collective buffers (per batch) ---------------------------
        ag_in = [dram.tile([QF, S], BF, name=f"agin{b}") for b in range(B)]
        ag_out = [dram.tile([TP * QF, S], BF, addr_space="Shared",
                            name=f"agout{b}") for b in range(B)]

        # ---- pools ----------------------------------------------------
        NTC = S // TOKC  # 4 chunks per batch
        xa_pool = ctx.enter_context(tc.tile_pool(name="xa", bufs=4))
        xt_pool = ctx.enter_context(tc.tile_pool(name="xt", bufs=30))
        qkv_pool = ctx.enter_context(tc.tile_pool(name="qkv", bufs=2))
        rope_pool = ctx.enter_context(tc.tile_pool(name="rope", bufs=1))
        v_pool = ctx.enter_context(tc.tile_pool(name="vtile", bufs=2 * (S // 128)))
        e_pool = ctx.enter_context(tc.tile_pool(name="epool", bufs=6))
        o_pool = ctx.enter_context(tc.tile_pool(name="opool", bufs=4))
        r_pool = ctx.enter_context(tc.tile_pool(name="rpool", bufs=4))
        wo_sbp = ctx.enter_context(tc.tile_pool(name="ag_sb", bufs=32))
        wo_out = ctx.enter_context(tc.tile_pool(name="wo_out", bufs=4))

        def wo_stage(bi):
            """out rows for batch bi from ag_out[bi] (needs that AG done)."""
            for tq in range(S // TOKC):
                agt = []
                for fb in range(NHB):
                    t = wo_sbp.tile([128, TOKC], BF, tag="agt",
                                    name=f"agt{bi}_{tq}_{fb}")
                    nc.sync.dma_start(
                        t[:], ag_out[bi][fb * 128:(fb + 1) * 128,
                                         tq * TOKC:(tq + 1) * TOKC])
                    agt.append(t)
                for tb in range(TOKC // 128):
                    psW = psum.tile([128, OC], F32, tag="o",
                                    name=f"psW{bi}_{tq}_{tb}")
                    for fb in range(NHB):
                        nc.tensor.matmul(
                            psW[:], agt[fb][:, tb * 128:(tb + 1) * 128],
                            wo_sb[fb][:], start=(fb == 0), stop=(fb == NHB - 1))
                    osb = wo_out.tile([128, OC], F32, tag="osb",
                                      name=f"osb{bi}_{tq}_{tb}")
                    nc.vector.tensor_copy(osb[:], psW[:])
                    row = bi * S + tq * TOKC + tb * 128
                    nc.sync.dma_start(out[row:row + 128, :], osb[:])

        for b in range(B):
            # -- QKV^T projection, token chunks of 512 --
            qt = [qkv_pool.tile([128, S], BF, tag=f"qt{i}", name=f"qt{i}")
                  for i in range(2)]
            kvT = qkv_pool.tile([128, S], BF, tag="kvT")
            kdup = qkv_pool.tile([128, S], BF, tag="kdup")
            for tcn in range(NTC):
                xa = []
                for tt in range(4):
                    t = xa_pool.tile([128, HID], BF, tag="xa",
                                     name=f"xa{b}_{tcn}_{tt}")
                    nc.gpsimd.dma_start(
                        t[:], x[b * S + tcn * TOKC + tt * 128:
                                b * S + tcn * TOKC + (tt + 1) * 128, :])
                    xa.append(t)
                xts = []
                for hb in range(NHB):
                    xt = xt_pool.tile([128, TOKC], BF, tag="xt",
                                      name=f"xt{b}_{tcn}_{hb}")
                    for tt in range(4):
                        nc.sync.dma_start_transpose(
                            xt[:, tt * 128:(tt + 1) * 128],
                            xa[tt][:, hb * 128:(hb + 1) * 128])
                    xts.append(xt)
                psq0 = psum.tile([128, TOKC], F32, tag="o", name=f"q0_{b}{tcn}")
                psq1 = psum.tile([128, TOKC], F32, tag="o", name=f"q1_{b}{tcn}")
                pskv = psum.tile([128, TOKC], F32, tag="o", name=f"kv_{b}{tcn}")
                for hb in range(NHB):
                    st, sp = hb == 0, hb == NHB - 1
                    nc.tensor.matmul(psq0[:], wq_sb[hb][:, 0:128], xts[hb][:],
                                     start=st, stop=sp)
                    nc.tensor.matmul(psq1[:], wq_sb[hb][:, 128:256], xts[hb][:],
                                     start=st, stop=sp)
                    nc.tensor.matmul(pskv[:], wkv_sb[hb][:], xts[hb][:],
                                     start=st, stop=sp)
                cs = slice(tcn * TOKC, (tcn + 1) * TOKC)
                nc.scalar.copy(qt[0][:, cs], psq0[:])
                nc.scalar.copy(qt[1][:, cs], psq1[:])
                nc.scalar.copy(kvT[:, cs], pskv[:])

            # -- RoPE on q (2 tiles, 2 heads each) and k --
            for qi in range(2):
                rot = rope_pool.tile([128, S], BF, tag="rot", name=f"rot{b}{qi}")
                for h2 in range(2):
                    o = h2 * 64
                    nc.gpsimd.dma_start(rot[o:o + 32, :], qt[qi][o + 32:o + 64, :])
                    nc.gpsimd.dma_start(rot[o + 32:o + 64, :], qt[qi][o:o + 32, :])
                tmp = rope_pool.tile([128, S], BF, tag="tmp", name=f"tmp{b}{qi}")
                nc.vector.tensor_mul(tmp[:], qt[qi][:], cosT[:])
                nc.vector.tensor_mul(rot[:], rot[:], sinTs[:])
                nc.vector.tensor_add(qt[qi][:], tmp[:], rot[:])
            rotk = rope_pool.tile([HD, S], BF, tag="rotk")
            nc.gpsimd.dma_start(rotk[0:32, :], kvT[32:HD, :])
            nc.gpsimd.dma_start(rotk[32:HD, :], kvT[0:32, :])
            tmpk = rope_pool.tile([HD, S], BF, tag="tmpk")
            nc.vector.tensor_mul(tmpk[:], kvT[0:HD, :], cosT[0:HD, :])
            nc.vector.tensor_mul(rotk[:], rotk[:], sinTs[0:HD, :])
            nc.vector.tensor_add(kvT[0:HD, :], tmpk[:], rotk[:])
            nc.gpsimd.dma_start(kdup[HD:128, :], kvT[0:HD, :])

            # -- V: transpose to token-major tiles [128, 65] (ones col) --
            vts = []
            for vb in range(S // 128):
                psv = psum.tile([128, HD], BF, tag="o", name=f"vps{b}_{vb}")
                nc.tensor.transpose(
                    psv[:], kvT[HD:128, vb * 128:(vb + 1) * 128],
                    id64hi[HD:128, :])
                vt_ = v_pool.tile([128, HD + 1], BF, tag="vt",
                                  name=f"vt{b}_{vb}")
                nc.scalar.copy(vt_[:, 0:HD], psv[:])
                nc.vector.memset(vt_[:, HD:HD + 1], 1.0)
                vts.append(vt_)

            # -- attention per head, q-chunks of 512, exp batched 1024 --
            for h in range(QH):
                r = h % 2
                qh_ap = qt[h // 2][r * 64:r * 64 + 64, :]
                k_src = kvT if r == 0 else kdup
                for qc in range(S // TOKC):
                    nkb = (qc + 1) * (TOKC // 128)
                    es = []  # (tile, col offset) per kb
                    for g in range(nkb // 2):
                        psS = psum_s.tile([128, 1024], F32, tag="s2",
                                        name=f"psS{b}{h}{qc}_{g}")
                        e = e_pool.tile([128, 1024], BF, tag="e",
                                        name=f"e{b}{h}{qc}_{g}")
                        for j in range(2):
                            kb = 2 * g + j
                            nc.tensor.matmul(
                                psS[:, j * TOKC:(j + 1) * TOKC],
                                k_src[r * 64:r * 64 + 64,
                                      kb * 128:(kb + 1) * 128],
                                qh_ap[:, qc * TOKC:(qc + 1) * TOKC],
                                start=True, stop=True)
                        nc.scalar.activation(
                            e[:], psS[:], mybir.ActivationFunctionType.Exp,
                            scale=0.125)
                        for j in range(2):
                            kb = 2 * g + j
                            if kb >= nkb - 4:
                                nc.gpsimd.affine_select(
                                    e[:, j * TOKC:(j + 1) * TOKC],
                                    e[:, j * TOKC:(j + 1) * TOKC],
                                    pattern=[[1, TOKC]],
                                    base=qc * TOKC - kb * 128,
                                    channel_multiplier=-1,
                                    compare_op=mybir.AluOpType.is_ge, fill=0.0)
                            es.append((e, j * TOKC))
                    psO = psum.tile([HD + 1, TOKC], F32, tag="o",
                                    name=f"psO{b}{h}{qc}")
                    for kb in range(nkb):
                        e, off = es[kb]
                        nc.tensor.matmul(psO[:], vts[kb][:],
                                         e[:, off:off + TOKC],
                                         start=(kb == 0), stop=(kb == nkb - 1))
                    srow = r_pool.tile([1, TOKC], F32, tag="srow",
                                       name=f"sr{b}{h}{qc}")
                    nc.vector.tensor_copy(srow[:], psO[HD:HD + 1, :])
                    recip = r_pool.tile([1, TOKC], F32, tag="recip",
                                        name=f"rc{b}{h}{qc}")
                    nc.vector.reciprocal_approx_fast(recip[:], srow[:])
                    recb = r_pool.tile([1, TOKC], BF, tag="recb",
                                       name=f"rb{b}{h}{qc}")
                    nc.vector.tensor_copy(recb[:], recip[:])
                    psB = psum.tile([HD, TOKC], F32, tag="o",
                                    name=f"psB{b}{h}{qc}")
                    nc.tensor.matmul(psB[:], ones_col[:], recb[:],
                                     start=True, stop=True)
                    bcs = o_pool.tile([HD, TOKC], BF, tag="bcs",
                                      name=f"bc{b}{h}{qc}")
                    nc.scalar.copy(bcs[:], psB[:])
                    ot = o_pool.tile([HD, TOKC], BF, tag="ot",
                                     name=f"ot{b}{h}{qc}")
                    nc.scalar.copy(ot[:], psO[0:HD, :])
                    at = o_pool.tile([HD, TOKC], BF, tag="at",
                                     name=f"at{b}{h}{qc}")
                    nc.vector.tensor_mul(at[:], ot[:], bcs[:])
                    nc.sync.dma_start(
                        ag_in[b][h * HD:(h + 1) * HD,
                                 qc * TOKC:(qc + 1) * TOKC],
                        at[:])

            # -- AllGather this batch's attention outputs --
            nc.gpsimd.collective_compute(
                "AllGather", mybir.AluOpType.bypass,
                ins=[ag_in[b][:].opt()], outs=[ag_out[b][:].opt()],
                replica_groups=[list(range(TP))],
            )
            if b == 0:
                wo_stage(0)
        wo_stage(1)

    nc.compile()
    return nc


def kernel(**inputs):
    global LAST_RESULTS, _NC_CACHE
    x = np.ascontiguousarray(inputs["x"].reshape(T, HID), dtype=np.float32)
    cos = np.ascontiguousarray(inputs["cos"], dtype=np.float32)
    sin = np.ascontiguousarray(inputs["sin"], dtype=np.float32)
    Wq = np.asarray(inputs["Wq"], dtype=np.float32)
    Wk = np.asarray(inputs["Wk"], dtype=np.float32)
    Wv = np.asarray(inputs["Wv"], dtype=np.float32)
    Wo = np.asarray(inputs["Wo"], dtype=np.float32)

    if _NC_CACHE is None:
        _NC_CACHE = build_nc()
    nc = _NC_CACHE

    in_maps = []
    for c in range(TP):
        in_maps.append({
            "x": x, "cos": cos, "sin": sin,
            "Wq": np.ascontiguousarray(Wq[:, c * QF:(c + 1) * QF]),
            "Wk": np.ascontiguousarray(Wk[:, c * HD:(c + 1) * HD]),
            "Wv": np.ascontiguousarray(Wv[:, c * HD:(c + 1) * HD]),
            "Wo": np.ascontiguousarray(Wo[:, c * OC:(c + 1) * OC]),
        })

    res = run_bass_kernel_spmd(nc, in_maps, core_ids=list(range(TP)))
    LAST_RESULTS = res
    full = np.concatenate([res.results[c]["out"] for c in range(TP)], axis=1)
    return np.ascontiguousarray(full.reshape(B, S, HID), dtype=np.float32)


if __name__ == "__main__":
    nc = build_nc()
    print("build OK, instructions:",
          sum(len(bb.instructions) for bb in nc.main_func.blocks))
